# revision 8
# baseline (speedup 1.0000x reference)
"""Trainium2 Bass kernel for nn_DRA_C_65644280152592 (dense_transformer).

Strategy: pure data-parallel over batch B=8 across 8 NeuronCores (one sample
per core). All matmul operands staged/cast to fp16 on host (PE runs fp16 at
full rate with fp32 PSUM accumulation); statistics, softmax, epilogues and
output in fp32.

Per-core pipeline (sample b):
  dec[512,112,112] resident in SBUF as fp16 (12.8 MB).
  Stage 1  patch embed: dlT[196,512] = X^T @ pe_w^T, X k-tiles are strided
           APs straight into the resident decoder (no data rearrangement);
           pe_w streamed from HBM as the moving operand. + pe_b via a K=1
           ones-row matmul.
  Stage 2  attention, transpose-free chain:
           km = trans@wk          [196,512]   (lhsT=transT staged on host)
           vT = wv^T@trans^T      [512,196]
           A  = dlT^T@km          [512,512]
           sim= wq^T@A            [512,512]  (s on partitions, t on free)
           InstanceNorm stats via row-reduce + ones-matmul partition reduce,
           softmax over free dim (exp on ACT with accum_out row sums),
           G  = sim_sm^T@wo as lhsT=sm  [512,512]
           recT = G^T@vT          [512,196]
           FIN = relu(rc'(recT)+b2')  [512,196]  (BN2 folded on host)
  Stage 3  mask conv fused: for each 4-row pixel block,
           out = relu(mc'(dec)+b1') * broadcast(FIN)   (BN1 folded on host)
"""
import sys

sys.path.insert(0, "/opt/trn_rl_repo")

import numpy as np

import concourse.bass as bass
import concourse.mybir as mybir
import concourse.tile as tile
from concourse import bacc
from concourse.bass_utils import run_bass_kernel_spmd

F16 = mybir.dt.float16
F32 = mybir.dt.float32
AF = mybir.ActivationFunctionType
AX = mybir.AxisListType
ALU = mybir.AluOpType

CIN, IMG, P = 512, 112, 8
NPR = 14                  # patches per side
NPAT = NPR * NPR          # 196
DEC = SKIP = 512
EMB = 768
BN_EPS = 1e-3
IN_EPS = 1e-3
N_CORES = 8
SIM_N = float(SKIP * SKIP)


def build_nc(repeat: int = 1, stages: int = 99):
    nc = bacc.Bacc(None)

    dec_d = nc.declare_dram_parameter("dec", [CIN, 64, NPAT], F16, isOutput=False)
    trT_d = nc.declare_dram_parameter("transT", [EMB, NPAT], F16, isOutput=False)
    pew_d = nc.declare_dram_parameter("pew", [256, 128, DEC], F16, isOutput=False)
    wq_d = nc.declare_dram_parameter("wq", [DEC, SKIP], F16, isOutput=False)
    wk_d = nc.declare_dram_parameter("wk", [EMB, SKIP], F16, isOutput=False)
    wv_d = nc.declare_dram_parameter("wv", [EMB, SKIP], F16, isOutput=False)
    wo_d = nc.declare_dram_parameter("wo", [SKIP, SKIP], F16, isOutput=False)
    mcT_d = nc.declare_dram_parameter("mcT", [CIN, SKIP], F16, isOutput=False)
    rcT_d = nc.declare_dram_parameter("rcT", [SKIP, SKIP], F16, isOutput=False)
    peb_d = nc.declare_dram_parameter("peb", [1, DEC], F16, isOutput=False)
    b1_d = nc.declare_dram_parameter("b1", [128, 4], F32, isOutput=False)
    b2_d = nc.declare_dram_parameter("b2", [128, 4], F32, isOutput=False)
    psi_d = nc.declare_dram_parameter("psi", [1, 2], F32, isOutput=False)
    out_d = nc.declare_dram_parameter("out", [SKIP, 64, NPAT], F32, isOutput=True)

    bc_scr = nc.dram_tensor("bc_scr", [1, 2], F32)
    bc_scr_ap = bc_scr[:]
    with tile.TileContext(nc) as tc:
        with tc.tile_pool(name="wts", bufs=1) as wts, \
             tc.tile_pool(name="pewp", bufs=4) as pewp, \
             tc.tile_pool(name="work", bufs=2) as work, \
             tc.tile_pool(name="st3", bufs=3) as st3, \
             tc.tile_pool(name="ph", bufs=1, space="PSUM") as ph, \
             tc.tile_pool(name="ps", bufs=4, space="PSUM") as ps:

            def body():
                # ---- resident loads ----
                ones16 = wts.tile([1, 128], F16, tag="ones16")
                nc.vector.memset(ones16, 1.0)

                peb = wts.tile([1, DEC], F16, tag="peb")
                nc.sync.dma_start(out=peb, in_=peb_d[:])
                b1 = wts.tile([128, 4], F32, tag="b1")
                nc.sync.dma_start(out=b1, in_=b1_d[:])
                b2 = wts.tile([128, 4], F32, tag="b2")
                nc.sync.dma_start(out=b2, in_=b2_d[:])
                psi = wts.tile([1, 2], F32, tag="psi")
                nc.sync.dma_start(out=psi, in_=psi_d[:])

                trT = wts.tile([128, 6, NPAT], F16, tag="trT")
                wk = wts.tile([128, 6, SKIP], F16, tag="wk")
                wv = wts.tile([128, 6, SKIP], F16, tag="wv")
                for kt in range(6):
                    nc.sync.dma_start(out=trT[:, kt, :],
                                      in_=trT_d[kt * 128:(kt + 1) * 128, :])
                    nc.sync.dma_start(out=wk[:, kt, :],
                                      in_=wk_d[kt * 128:(kt + 1) * 128, :])
                    nc.sync.dma_start(out=wv[:, kt, :],
                                      in_=wv_d[kt * 128:(kt + 1) * 128, :])
                wq = wts.tile([128, 4, SKIP], F16, tag="wq")
                wo = wts.tile([128, 4, SKIP], F16, tag="wo")
                mcT = wts.tile([128, 4, SKIP], F16, tag="mcT")
                rcT = wts.tile([128, 4, SKIP], F16, tag="rcT")
                for kt in range(4):
                    nc.sync.dma_start(out=wq[:, kt, :],
                                      in_=wq_d[kt * 128:(kt + 1) * 128, :])
                    nc.sync.dma_start(out=wo[:, kt, :],
                                      in_=wo_d[kt * 128:(kt + 1) * 128, :])
                    nc.sync.dma_start(out=mcT[:, kt, :],
                                      in_=mcT_d[kt * 128:(kt + 1) * 128, :])
                    nc.sync.dma_start(out=rcT[:, kt, :],
                                      in_=rcT_d[kt * 128:(kt + 1) * 128, :])

                dec_sb = []
                for cb in range(4):
                    t = wts.tile([128, 64, NPAT], F16, tag=f"dec{cb}")
                    nc.sync.dma_start(out=t, in_=dec_d[cb * 128:(cb + 1) * 128, :, :])
                    dec_sb.append(t)

                if stages < 1:
                    return
                # ---- early attention matmuls (only need trans + wk/wv) ----
                # km[n,s] = sum_e trans[n,e] wk[e,s] ; two M halves of 98
                km = [wts.tile([98, SKIP], F16, tag=f"km{h}", name=f"km{h}")
                      for h in range(2)]
                for h in range(2):
                    pk = ps.tile([98, SKIP], F32, tag="pt")
                    for kt in range(6):
                        nc.tensor.matmul(pk, trT[:, kt, h * 98:(h + 1) * 98],
                                         wk[:, kt, :],
                                         start=(kt == 0), stop=(kt == 5))
                    nc.scalar.copy(km[h], pk)

                # vT[t,n] = sum_e wv[e,t] trans[n,e]  -> [512,196]
                vT = wts.tile([128, 4, NPAT], F16, tag="vT")
                for m in range(4):
                    pv = ps.tile([128, NPAT], F32, tag="pt")
                    for kt in range(6):
                        nc.tensor.matmul(pv, wv[:, kt, m * 128:(m + 1) * 128],
                                         trT[:, kt, :],
                                         start=(kt == 0), stop=(kt == 5))
                    nc.scalar.copy(vT[:, m, :], pv)

                if stages < 2:
                    return
                # ---- stage 1: patch embedding ----
                # dlT[n,d] = sum_{c,py,px} dec[c, 8pr+py, 8pc+px] pew[(py,px,c),d]
                pdl = [ph.tile([98, DEC], F32, tag=f"pdl{h}", name=f"pdl{h}")
                       for h in range(2)]
                NK = 256
                for py in range(8):
                    for px in range(8):
                        for cb in range(4):
                            k = (py * 8 + px) * 4 + cb
                            pw = pewp.tile([128, DEC], F16, tag="pw")
                            nc.sync.dma_start(out=pw, in_=pew_d[k, :, :])
                            for h in range(2):
                                xs = dec_sb[cb][:, py * 8 + px,
                                                98 * h:98 * (h + 1)]
                                nc.tensor.matmul(pdl[h], xs, pw,
                                                 start=(k == 0), stop=False)
                dlT = [wts.tile([98, DEC], F16, tag=f"dlT{h}", name=f"dlT{h}")
                       for h in range(2)]
                for h in range(2):
                    nc.tensor.matmul(pdl[h], ones16[:1, :98], peb,
                                     start=False, stop=True)
                    nc.scalar.copy(dlT[h], pdl[h])

                if stages < 3:
                    return
                # ---- stage 2: attention ----
                # A[d,t] = sum_n dlT[n,d] km[n,t]
                A = wts.tile([128, 4, SKIP], F16, tag="A")
                for m in range(4):
                    pa = ps.tile([128, SKIP], F32, tag="pt")
                    for h in range(2):
                        nc.tensor.matmul(pa, dlT[h][:, m * 128:(m + 1) * 128],
                                         km[h], start=(h == 0), stop=(h == 1))
                    nc.scalar.copy(A[:, m, :], pa)

                # sim[s,t] = sum_d wq[d,s] A[d,t]
                simf = wts.tile([128, 4, SKIP], F32, tag="simf")
                for m in range(4):
                    pc = ps.tile([128, SKIP], F32, tag="pt")
                    for kt in range(4):
                        nc.tensor.matmul(pc, wq[:, kt, m * 128:(m + 1) * 128],
                                         A[:, kt, :],
                                         start=(kt == 0), stop=(kt == 3))
                    nc.scalar.copy(simf[:, m, :], pc)

                if stages < 4:
                    return
                # instance-norm stats over the whole 512x512 map
                statp = wts.tile([128, 8], F32, tag="statp")
                sqs = work.tile([128, SKIP], F32, tag="sqs")
                for m in range(4):
                    nc.vector.reduce_sum(statp[:, m:m + 1], simf[:, m, :], axis=AX.X)
                    nc.scalar.square(sqs, simf[:, m, :])
                    nc.vector.reduce_sum(statp[:, 4 + m:5 + m], sqs, axis=AX.X)
                srow = wts.tile([128, 2], F32, tag="srow")
                nc.vector.reduce_sum(srow[:, 0:1], statp[:, 0:4], axis=AX.X)
                nc.vector.reduce_sum(srow[:, 1:2], statp[:, 4:8], axis=AX.X)
                # partition -> free flip via tiny SBUF-to-SBUF DMA, then reduce
                flip = wts.tile([1, 2, 128], F32, tag="flip")
                for j in range(2):
                    nc.sync.dma_start(out=flip[:, j, :], in_=srow[:, j:j + 1])
                # scalars on partition 0
                sc = wts.tile([1, 8], F32, tag="sc")
                # sc cols: 0=s,1=q,2=mu,3=ex2,4=musq,5=var,6=sqrt,7=rsig
                epsT = wts.tile([1, 1], F32, tag="epsT")
                nc.vector.memset(epsT, IN_EPS)
                nc.vector.reduce_sum(sc[:, 0:1], flip[:, 0, :], axis=AX.X)
                nc.vector.reduce_sum(sc[:, 1:2], flip[:, 1, :], axis=AX.X)
                nc.scalar.mul(sc[:, 2:3], sc[:, 0:1], 1.0 / SIM_N)
                nc.scalar.mul(sc[:, 3:4], sc[:, 1:2], 1.0 / SIM_N)
                nc.vector.tensor_mul(sc[:, 4:5], sc[:, 2:3], sc[:, 2:3])
                nc.vector.tensor_sub(sc[:, 5:6], sc[:, 3:4], sc[:, 4:5])
                nc.scalar.activation(sc[:, 6:7], sc[:, 5:6], AF.Sqrt, bias=epsT)
                nc.vector.reciprocal(sc[:, 7:8], sc[:, 6:7])
                scal2 = wts.tile([1, 2], F32, tag="scal2")
                nc.vector.tensor_mul(scal2[:, 0:1], sc[:, 7:8], psi[:, 0:1])
                nc.scalar.mul(scal2[:, 1:2], scal2[:, 0:1], -1.0)
                # broadcast to all partitions via DRAM bounce
                nc.sync.dma_start(out=bc_scr_ap, in_=scal2)
                bcast_in = bass.AP(tensor=bc_scr_ap.tensor, offset=bc_scr_ap.offset,
                                   ap=[[0, 128], [1, 2]])
                bc = wts.tile([128, 2], F32, tag="bc")
                nc.sync.dma_start(out=bc, in_=bcast_in)

                if stages < 5:
                    return
                # softmax over t (free dim); psi_b cancels in softmax
                sm16 = wts.tile([128, 4, SKIP], F16, tag="sm16")
                for m in range(4):
                    rmax = work.tile([128, 1], F32, tag="rmax")
                    nc.vector.reduce_max(rmax, simf[:, m, :], axis=AX.X)
                    nm = work.tile([128, 1], F32, tag="nm")
                    nc.vector.tensor_mul(nm, rmax, bc[:, 1:2])
                    rsum = work.tile([128, 1], F32, tag="rsum")
                    nc.scalar.activation(simf[:, m, :], simf[:, m, :], AF.Exp,
                                         bias=nm, scale=bc[:, 0:1],
                                         accum_out=rsum)
                    rinv = work.tile([128, 1], F32, tag="rinv")
                    nc.vector.reciprocal(rinv, rsum)
                    nc.vector.tensor_scalar_mul(sm16[:, m, :], simf[:, m, :], rinv)

                if stages < 6:
                    return
                # G[t,o] = sum_s sm[s,t] wo[s,o]
                G = wts.tile([128, 4, SKIP], F16, tag="G")
                for m in range(4):
                    pg = ps.tile([128, SKIP], F32, tag="pt")
                    for kt in range(4):
                        nc.tensor.matmul(pg, sm16[:, kt, m * 128:(m + 1) * 128],
                                         wo[:, kt, :],
                                         start=(kt == 0), stop=(kt == 3))
                    nc.scalar.copy(G[:, m, :], pg)

                # recT[o,n] = sum_t G[t,o] vT[t,n]
                recT = wts.tile([128, 4, NPAT], F16, tag="recT")
                for m in range(4):
                    pr_ = ps.tile([128, NPAT], F32, tag="pt")
                    for kt in range(4):
                        nc.tensor.matmul(pr_, G[:, kt, m * 128:(m + 1) * 128],
                                         vT[:, kt, :],
                                         start=(kt == 0), stop=(kt == 3))
                    nc.scalar.copy(recT[:, m, :], pr_)

                # FIN = relu(rc'(recT) + b2')
                FIN = wts.tile([128, 4, NPAT], F32, tag="FIN")
                for m in range(4):
                    pf = ps.tile([128, NPAT], F32, tag="pt")
                    for kt in range(4):
                        nc.tensor.matmul(pf, rcT[:, kt, m * 128:(m + 1) * 128],
                                         recT[:, kt, :],
                                         start=(kt == 0), stop=(kt == 3))
                    nc.scalar.activation(FIN[:, m, :], pf, AF.Relu,
                                         bias=b2[:, m:m + 1])

                if stages < 7:
                    return
                # ---- stage 3: mask conv + recon multiply (patch-major) ----
                for pp in range(64):
                    for m in range(4):
                        pM = ps.tile([128, NPAT], F32, tag="pt")
                        for kt in range(4):
                            nc.tensor.matmul(pM,
                                             mcT[:, kt, m * 128:(m + 1) * 128],
                                             dec_sb[kt][:, pp, :],
                                             start=(kt == 0), stop=(kt == 3))
                        rl = st3.tile([128, NPAT], F32, tag="rl", bufs=6)
                        nc.scalar.activation(rl, pM, AF.Relu, bias=b1[:, m:m + 1])
                        ot = st3.tile([128, NPAT], F32, tag="ot", bufs=6)
                        nc.vector.tensor_mul(ot, rl, FIN[:, m, :])
                        nc.sync.dma_start(
                            out=out_d[m * 128:(m + 1) * 128, pp, :],
                            in_=ot)

            if repeat == 1:
                body()
            else:
                with tc.For_i(0, repeat, 1):
                    body()
    nc.finalize()
    return nc


def prepare_in_maps(inputs: dict) -> list[dict]:
    f16 = np.float16
    decoder = np.asarray(inputs["decoder"], np.float32)
    trans = np.asarray(inputs["trans"], np.float32)
    pe_w = np.asarray(inputs["pe_w"], np.float32)
    pe_b = np.asarray(inputs["pe_b"], np.float32)
    mc_w = np.asarray(inputs["mc_w"], np.float32)
    mc_b = np.asarray(inputs["mc_b"], np.float32)
    bn1_g = np.asarray(inputs["bn1_g"], np.float32)
    bn1_b = np.asarray(inputs["bn1_b"], np.float32)
    bn1_m = np.asarray(inputs["bn1_m"], np.float32)
    bn1_v = np.asarray(inputs["bn1_v"], np.float32)
    wq = np.asarray(inputs["wq"], np.float32)
    wk = np.asarray(inputs["wk"], np.float32)
    wv = np.asarray(inputs["wv"], np.float32)
    wo = np.asarray(inputs["wo"], np.float32)
    psi_g = np.asarray(inputs["psi_g"], np.float32)
    psi_b = np.asarray(inputs["psi_b"], np.float32)
    rc_w = np.asarray(inputs["rc_w"], np.float32)
    rc_b = np.asarray(inputs["rc_b"], np.float32)
    bn2_g = np.asarray(inputs["bn2_g"], np.float32)
    bn2_b = np.asarray(inputs["bn2_b"], np.float32)
    bn2_m = np.asarray(inputs["bn2_m"], np.float32)
    bn2_v = np.asarray(inputs["bn2_v"], np.float32)

    s1 = bn1_g / np.sqrt(bn1_v + BN_EPS)
    mcT = np.ascontiguousarray((mc_w[:, :, 0, 0] * s1[:, None]).T)
    b1 = (mc_b - bn1_m) * s1 + bn1_b
    s2 = bn2_g / np.sqrt(bn2_v + BN_EPS)
    rcT = np.ascontiguousarray((rc_w[:, :, 0, 0] * s2[:, None]).T)
    b2 = (rc_b - bn2_m) * s2 + bn2_b

    pew = np.ascontiguousarray(pe_w.transpose(2, 3, 1, 0)).reshape(256, 128, DEC)

    shared = {
        "pew": pew.astype(f16),
        "wq": wq.astype(f16),
        "wk": wk.astype(f16),
        "wv": wv.astype(f16),
        "wo": wo.astype(f16),
        "mcT": mcT.astype(f16),
        "rcT": rcT.astype(f16),
        "peb": pe_b.reshape(1, DEC).astype(f16),
        "b1": np.ascontiguousarray(b1.reshape(4, 128).T).astype(np.float32),
        "b2": np.ascontiguousarray(b2.reshape(4, 128).T).astype(np.float32),
        "psi": np.array([[psi_g[0], psi_b[0]]], np.float32),
    }
    in_maps = []
    for c in range(N_CORES):
        m = dict(shared)
        m["dec"] = np.ascontiguousarray(
            decoder[c].reshape(CIN, NPR, P, NPR, P).transpose(0, 2, 4, 1, 3)
            .reshape(CIN, 64, NPAT)).astype(f16)
        m["transT"] = np.ascontiguousarray(trans[c].T).astype(f16)
        in_maps.append(m)
    return in_maps


_NC_CACHE: dict = {}


def get_nc(repeat: int = 1):
    if repeat not in _NC_CACHE:
        _NC_CACHE[repeat] = build_nc(repeat)
    return _NC_CACHE[repeat]


def kernel(**inputs) -> np.ndarray:
    nc = get_nc(1)
    in_maps = prepare_in_maps(inputs)
    res = run_bass_kernel_spmd(nc, in_maps, core_ids=list(range(N_CORES)))
    outs = []
    for c in range(N_CORES):
        oq = res.results[c]["out"].reshape(SKIP, P, P, NPR, NPR)
        outs.append(oq.transpose(0, 3, 1, 4, 2).reshape(SKIP, IMG, IMG))
    return np.stack(outs).astype(np.float32)


if __name__ == "__main__":
    import jax

    sys.path.insert(0, "/root/problem")
    import reference

    inputs = {k: np.asarray(v) for k, v in reference.setup_inputs().items()}
    out = kernel(**inputs)
    print("out shape", out.shape, out.dtype)


# revision 9
# speedup vs baseline: 14.1549x; 14.1549x over previous
"""Trainium2 Bass kernel for nn_DRA_C_65644280152592 (dense_transformer).

Strategy: pure data-parallel over batch B=8 across 8 NeuronCores (one sample
per core). All matmul operands staged/cast to fp16 on host (PE runs fp16 at
full rate with fp32 PSUM accumulation); statistics, softmax, epilogues and
output in fp32.

Per-core pipeline (sample b):
  dec[512,112,112] resident in SBUF as fp16 (12.8 MB).
  Stage 1  patch embed: dlT[196,512] = X^T @ pe_w^T, X k-tiles are strided
           APs straight into the resident decoder (no data rearrangement);
           pe_w streamed from HBM as the moving operand. + pe_b via a K=1
           ones-row matmul.
  Stage 2  attention, transpose-free chain:
           km = trans@wk          [196,512]   (lhsT=transT staged on host)
           vT = wv^T@trans^T      [512,196]
           A  = dlT^T@km          [512,512]
           sim= wq^T@A            [512,512]  (s on partitions, t on free)
           InstanceNorm stats via row-reduce + ones-matmul partition reduce,
           softmax over free dim (exp on ACT with accum_out row sums),
           G  = sim_sm^T@wo as lhsT=sm  [512,512]
           recT = G^T@vT          [512,196]
           FIN = relu(rc'(recT)+b2')  [512,196]  (BN2 folded on host)
  Stage 3  mask conv fused: for each 4-row pixel block,
           out = relu(mc'(dec)+b1') * broadcast(FIN)   (BN1 folded on host)
"""
import sys

sys.path.insert(0, "/opt/trn_rl_repo")

import numpy as np

import concourse.bass as bass
import concourse.mybir as mybir
import concourse.tile as tile
from concourse import bacc
from concourse.bass_utils import run_bass_kernel_spmd

F16 = mybir.dt.float16
F32 = mybir.dt.float32
AF = mybir.ActivationFunctionType
AX = mybir.AxisListType
ALU = mybir.AluOpType

CIN, IMG, P = 512, 112, 8
NPR = 14                  # patches per side
NPAT = NPR * NPR          # 196
DEC = SKIP = 512
EMB = 768
BN_EPS = 1e-3
IN_EPS = 1e-3
N_CORES = 8
SIM_N = float(SKIP * SKIP)


def build_nc(repeat: int = 1, stages: int = 99):
    nc = bacc.Bacc(None)

    dec_d = nc.declare_dram_parameter("dec", [CIN, 64, NPAT], F16, isOutput=False)
    trT_d = nc.declare_dram_parameter("transT", [EMB, NPAT], F16, isOutput=False)
    pew_d = nc.declare_dram_parameter("pew", [256, 128, DEC], F16, isOutput=False)
    wq_d = nc.declare_dram_parameter("wq", [DEC, SKIP], F16, isOutput=False)
    wk_d = nc.declare_dram_parameter("wk", [EMB, SKIP], F16, isOutput=False)
    wv_d = nc.declare_dram_parameter("wv", [EMB, SKIP], F16, isOutput=False)
    wo_d = nc.declare_dram_parameter("wo", [SKIP, SKIP], F16, isOutput=False)
    mcT_d = nc.declare_dram_parameter("mcT", [CIN, SKIP], F16, isOutput=False)
    rcT_d = nc.declare_dram_parameter("rcT", [SKIP, SKIP], F16, isOutput=False)
    peb_d = nc.declare_dram_parameter("peb", [1, DEC], F16, isOutput=False)
    b1_d = nc.declare_dram_parameter("b1", [128, 4], F32, isOutput=False)
    b2_d = nc.declare_dram_parameter("b2", [128, 4], F32, isOutput=False)
    psi_d = nc.declare_dram_parameter("psi", [1, 2], F32, isOutput=False)
    out_d = nc.declare_dram_parameter("out", [SKIP, 64, NPAT], F32, isOutput=True)

    bc_scr = nc.dram_tensor("bc_scr", [1, 2], F32)
    bc_scr_ap = bc_scr[:]
    with tile.TileContext(nc) as tc:
        with tc.tile_pool(name="wts", bufs=1) as wts, \
             tc.tile_pool(name="pewp", bufs=8) as pewp, \
             tc.tile_pool(name="work", bufs=2) as work, \
             tc.tile_pool(name="st3", bufs=3) as st3, \
             tc.tile_pool(name="ph", bufs=1, space="PSUM") as ph, \
             tc.tile_pool(name="ps", bufs=4, space="PSUM") as ps:

            def body():
                # ---- resident loads ----
                ones16 = wts.tile([1, 128], F16, tag="ones16")
                nc.vector.memset(ones16, 1.0)

                peb = wts.tile([1, DEC], F16, tag="peb")
                nc.sync.dma_start(out=peb, in_=peb_d[:])
                b1 = wts.tile([128, 4], F32, tag="b1")
                nc.sync.dma_start(out=b1, in_=b1_d[:])
                b2 = wts.tile([128, 4], F32, tag="b2")
                nc.sync.dma_start(out=b2, in_=b2_d[:])
                psi = wts.tile([1, 2], F32, tag="psi")
                nc.sync.dma_start(out=psi, in_=psi_d[:])

                trT = wts.tile([128, 6, NPAT], F16, tag="trT")
                wk = wts.tile([128, 6, SKIP], F16, tag="wk")
                wv = wts.tile([128, 6, SKIP], F16, tag="wv")
                for kt in range(6):
                    nc.sync.dma_start(out=trT[:, kt, :],
                                      in_=trT_d[kt * 128:(kt + 1) * 128, :])
                    nc.sync.dma_start(out=wk[:, kt, :],
                                      in_=wk_d[kt * 128:(kt + 1) * 128, :])
                    nc.sync.dma_start(out=wv[:, kt, :],
                                      in_=wv_d[kt * 128:(kt + 1) * 128, :])
                wq = wts.tile([128, 4, SKIP], F16, tag="wq")
                wo = wts.tile([128, 4, SKIP], F16, tag="wo")
                mcT = wts.tile([128, 4, SKIP], F16, tag="mcT")
                rcT = wts.tile([128, 4, SKIP], F16, tag="rcT")
                for kt in range(4):
                    nc.sync.dma_start(out=wq[:, kt, :],
                                      in_=wq_d[kt * 128:(kt + 1) * 128, :])
                    nc.sync.dma_start(out=wo[:, kt, :],
                                      in_=wo_d[kt * 128:(kt + 1) * 128, :])
                    nc.sync.dma_start(out=mcT[:, kt, :],
                                      in_=mcT_d[kt * 128:(kt + 1) * 128, :])
                    nc.sync.dma_start(out=rcT[:, kt, :],
                                      in_=rcT_d[kt * 128:(kt + 1) * 128, :])

                dec_sb = []
                for cb in range(4):
                    t = wts.tile([128, 64 * NPAT], F16, tag=f"dec{cb}")
                    nc.sync.dma_start(
                        out=t.rearrange("p (a b) -> p a b", b=NPAT),
                        in_=dec_d[cb * 128:(cb + 1) * 128, :, :])
                    dec_sb.append(t)

                if stages < 1:
                    return
                # ---- early attention matmuls (only need trans + wk/wv) ----
                # km[n,s] = sum_e trans[n,e] wk[e,s] ; two M halves of 98
                km = [wts.tile([98, SKIP], F16, tag=f"km{h}", name=f"km{h}")
                      for h in range(2)]
                for h in range(2):
                    pk = ps.tile([98, SKIP], F32, tag="pt")
                    for kt in range(6):
                        nc.tensor.matmul(pk, trT[:, kt, h * 98:(h + 1) * 98],
                                         wk[:, kt, :],
                                         start=(kt == 0), stop=(kt == 5))
                    nc.scalar.copy(km[h], pk)

                # vT[t,n] = sum_e wv[e,t] trans[n,e]  -> [512,196]
                vT = wts.tile([128, 4, NPAT], F16, tag="vT")
                for m in range(4):
                    pv = ps.tile([128, NPAT], F32, tag="pt")
                    for kt in range(6):
                        nc.tensor.matmul(pv, wv[:, kt, m * 128:(m + 1) * 128],
                                         trT[:, kt, :],
                                         start=(kt == 0), stop=(kt == 5))
                    nc.scalar.copy(vT[:, m, :], pv)

                if stages < 2:
                    return
                # ---- stage 1: patch embedding ----
                # dlT[n,d] = sum_{c,py,px} dec[c, 8pr+py, 8pc+px] pew[(py,px,c),d]
                pdl = [ph.tile([98, DEC], F32, tag=f"pdl{h}", name=f"pdl{h}")
                       for h in range(2)]
                for cb in range(4):
                    for pp in range(64):
                        k = cb * 64 + pp
                        pw = pewp.tile([128, DEC], F16, tag="pw")
                        nc.sync.dma_start(out=pw, in_=pew_d[k, :, :])
                        for h in range(2):
                            xs = dec_sb[cb][:, pp * NPAT + 98 * h:
                                            pp * NPAT + 98 * (h + 1)]
                            nc.tensor.matmul(pdl[h], xs, pw,
                                             start=(k == 0), stop=False)
                dlT = [wts.tile([98, DEC], F16, tag=f"dlT{h}", name=f"dlT{h}")
                       for h in range(2)]
                for h in range(2):
                    nc.tensor.matmul(pdl[h], ones16[:1, :98], peb,
                                     start=False, stop=True)
                    nc.scalar.copy(dlT[h], pdl[h])

                if stages < 3:
                    return
                # ---- stage 2: attention ----
                # A[d,t] = sum_n dlT[n,d] km[n,t]
                A = wts.tile([128, 4, SKIP], F16, tag="A")
                for m in range(4):
                    pa = ps.tile([128, SKIP], F32, tag="pt")
                    for h in range(2):
                        nc.tensor.matmul(pa, dlT[h][:, m * 128:(m + 1) * 128],
                                         km[h], start=(h == 0), stop=(h == 1))
                    nc.scalar.copy(A[:, m, :], pa)

                # sim[s,t] = sum_d wq[d,s] A[d,t]
                simf = wts.tile([128, 4, SKIP], F32, tag="simf")
                for m in range(4):
                    pc = ps.tile([128, SKIP], F32, tag="pt")
                    for kt in range(4):
                        nc.tensor.matmul(pc, wq[:, kt, m * 128:(m + 1) * 128],
                                         A[:, kt, :],
                                         start=(kt == 0), stop=(kt == 3))
                    nc.scalar.copy(simf[:, m, :], pc)

                if stages < 4:
                    return
                # instance-norm stats over the whole 512x512 map
                statp = wts.tile([128, 8], F32, tag="statp")
                sqs = work.tile([128, SKIP], F32, tag="sqs")
                for m in range(4):
                    nc.vector.reduce_sum(statp[:, m:m + 1], simf[:, m, :], axis=AX.X)
                    nc.scalar.square(sqs, simf[:, m, :])
                    nc.vector.reduce_sum(statp[:, 4 + m:5 + m], sqs, axis=AX.X)
                srow = wts.tile([128, 2], F32, tag="srow")
                nc.vector.reduce_sum(srow[:, 0:1], statp[:, 0:4], axis=AX.X)
                nc.vector.reduce_sum(srow[:, 1:2], statp[:, 4:8], axis=AX.X)
                # partition -> free flip via tiny SBUF-to-SBUF DMA, then reduce
                flip = wts.tile([1, 2, 128], F32, tag="flip")
                for j in range(2):
                    nc.sync.dma_start(out=flip[:, j, :], in_=srow[:, j:j + 1])
                # scalars on partition 0
                sc = wts.tile([1, 8], F32, tag="sc")
                # sc cols: 0=s,1=q,2=mu,3=ex2,4=musq,5=var,6=sqrt,7=rsig
                epsT = wts.tile([1, 1], F32, tag="epsT")
                nc.vector.memset(epsT, IN_EPS)
                nc.vector.reduce_sum(sc[:, 0:1], flip[:, 0, :], axis=AX.X)
                nc.vector.reduce_sum(sc[:, 1:2], flip[:, 1, :], axis=AX.X)
                nc.scalar.mul(sc[:, 2:3], sc[:, 0:1], 1.0 / SIM_N)
                nc.scalar.mul(sc[:, 3:4], sc[:, 1:2], 1.0 / SIM_N)
                nc.vector.tensor_mul(sc[:, 4:5], sc[:, 2:3], sc[:, 2:3])
                nc.vector.tensor_sub(sc[:, 5:6], sc[:, 3:4], sc[:, 4:5])
                nc.scalar.activation(sc[:, 6:7], sc[:, 5:6], AF.Sqrt, bias=epsT)
                nc.vector.reciprocal(sc[:, 7:8], sc[:, 6:7])
                scal2 = wts.tile([1, 2], F32, tag="scal2")
                nc.vector.tensor_mul(scal2[:, 0:1], sc[:, 7:8], psi[:, 0:1])
                nc.scalar.mul(scal2[:, 1:2], scal2[:, 0:1], -1.0)
                # broadcast to all partitions via DRAM bounce
                nc.sync.dma_start(out=bc_scr_ap, in_=scal2)
                bcast_in = bass.AP(tensor=bc_scr_ap.tensor, offset=bc_scr_ap.offset,
                                   ap=[[0, 128], [1, 2]])
                bc = wts.tile([128, 2], F32, tag="bc")
                nc.sync.dma_start(out=bc, in_=bcast_in)

                if stages < 5:
                    return
                # softmax over t (free dim); psi_b cancels in softmax
                sm16 = wts.tile([128, 4, SKIP], F16, tag="sm16")
                for m in range(4):
                    rmax = work.tile([128, 1], F32, tag="rmax")
                    nc.vector.reduce_max(rmax, simf[:, m, :], axis=AX.X)
                    nm = work.tile([128, 1], F32, tag="nm")
                    nc.vector.tensor_mul(nm, rmax, bc[:, 1:2])
                    rsum = work.tile([128, 1], F32, tag="rsum")
                    nc.scalar.activation(simf[:, m, :], simf[:, m, :], AF.Exp,
                                         bias=nm, scale=bc[:, 0:1],
                                         accum_out=rsum)
                    rinv = work.tile([128, 1], F32, tag="rinv")
                    nc.vector.reciprocal(rinv, rsum)
                    nc.vector.tensor_scalar_mul(sm16[:, m, :], simf[:, m, :], rinv)

                if stages < 6:
                    return
                # G[t,o] = sum_s sm[s,t] wo[s,o]
                G = wts.tile([128, 4, SKIP], F16, tag="G")
                for m in range(4):
                    pg = ps.tile([128, SKIP], F32, tag="pt")
                    for kt in range(4):
                        nc.tensor.matmul(pg, sm16[:, kt, m * 128:(m + 1) * 128],
                                         wo[:, kt, :],
                                         start=(kt == 0), stop=(kt == 3))
                    nc.scalar.copy(G[:, m, :], pg)

                # recT[o,n] = sum_t G[t,o] vT[t,n]
                recT = wts.tile([128, 4, NPAT], F16, tag="recT")
                for m in range(4):
                    pr_ = ps.tile([128, NPAT], F32, tag="pt")
                    for kt in range(4):
                        nc.tensor.matmul(pr_, G[:, kt, m * 128:(m + 1) * 128],
                                         vT[:, kt, :],
                                         start=(kt == 0), stop=(kt == 3))
                    nc.scalar.copy(recT[:, m, :], pr_)

                # FIN = relu(rc'(recT) + b2')
                FIN = wts.tile([128, 4, NPAT], F32, tag="FIN")
                for m in range(4):
                    pf = ps.tile([128, NPAT], F32, tag="pt")
                    for kt in range(4):
                        nc.tensor.matmul(pf, rcT[:, kt, m * 128:(m + 1) * 128],
                                         recT[:, kt, :],
                                         start=(kt == 0), stop=(kt == 3))
                    nc.scalar.activation(FIN[:, m, :], pf, AF.Relu,
                                         bias=b2[:, m:m + 1])

                if stages < 7:
                    return
                # ---- stage 3: mask conv + recon multiply (patch-major) ----
                out_flat = out_d.rearrange("c a b -> c (a b)")
                W3 = 2 * NPAT
                for ppb in range(32):
                    p0 = ppb * W3
                    for m in range(4):
                        pM = ps.tile([128, W3], F32, tag="pt")
                        for kt in range(4):
                            nc.tensor.matmul(pM,
                                             mcT[:, kt, m * 128:(m + 1) * 128],
                                             dec_sb[kt][:, p0:p0 + W3],
                                             start=(kt == 0), stop=(kt == 3))
                        rl = st3.tile([128, W3], F32, tag="rl", bufs=6)
                        nc.scalar.activation(rl, pM, AF.Relu, bias=b1[:, m:m + 1])
                        ot = st3.tile([128, W3], F32, tag="ot", bufs=6)
                        fbase = FIN[:, m, :]
                        fb = bass.AP(tensor=fbase.tensor, offset=fbase.offset,
                                     ap=[fbase.ap[0], [0, 2], fbase.ap[1]])
                        nc.vector.tensor_mul(
                            ot.rearrange("p (a b) -> p a b", b=NPAT),
                            rl.rearrange("p (a b) -> p a b", b=NPAT), fb)
                        nc.sync.dma_start(
                            out=out_flat[m * 128:(m + 1) * 128, p0:p0 + W3],
                            in_=ot)

            if repeat == 1:
                body()
            else:
                with tc.For_i(0, repeat, 1):
                    body()
    nc.finalize()
    return nc


def prepare_in_maps(inputs: dict) -> list[dict]:
    f16 = np.float16
    decoder = np.asarray(inputs["decoder"], np.float32)
    trans = np.asarray(inputs["trans"], np.float32)
    pe_w = np.asarray(inputs["pe_w"], np.float32)
    pe_b = np.asarray(inputs["pe_b"], np.float32)
    mc_w = np.asarray(inputs["mc_w"], np.float32)
    mc_b = np.asarray(inputs["mc_b"], np.float32)
    bn1_g = np.asarray(inputs["bn1_g"], np.float32)
    bn1_b = np.asarray(inputs["bn1_b"], np.float32)
    bn1_m = np.asarray(inputs["bn1_m"], np.float32)
    bn1_v = np.asarray(inputs["bn1_v"], np.float32)
    wq = np.asarray(inputs["wq"], np.float32)
    wk = np.asarray(inputs["wk"], np.float32)
    wv = np.asarray(inputs["wv"], np.float32)
    wo = np.asarray(inputs["wo"], np.float32)
    psi_g = np.asarray(inputs["psi_g"], np.float32)
    psi_b = np.asarray(inputs["psi_b"], np.float32)
    rc_w = np.asarray(inputs["rc_w"], np.float32)
    rc_b = np.asarray(inputs["rc_b"], np.float32)
    bn2_g = np.asarray(inputs["bn2_g"], np.float32)
    bn2_b = np.asarray(inputs["bn2_b"], np.float32)
    bn2_m = np.asarray(inputs["bn2_m"], np.float32)
    bn2_v = np.asarray(inputs["bn2_v"], np.float32)

    s1 = bn1_g / np.sqrt(bn1_v + BN_EPS)
    mcT = np.ascontiguousarray((mc_w[:, :, 0, 0] * s1[:, None]).T)
    b1 = (mc_b - bn1_m) * s1 + bn1_b
    s2 = bn2_g / np.sqrt(bn2_v + BN_EPS)
    rcT = np.ascontiguousarray((rc_w[:, :, 0, 0] * s2[:, None]).T)
    b2 = (rc_b - bn2_m) * s2 + bn2_b

    pew = np.ascontiguousarray(
        pe_w.transpose(1, 2, 3, 0).reshape(4, 128, 64, DEC).transpose(0, 2, 1, 3)
    ).reshape(256, 128, DEC)

    shared = {
        "pew": pew.astype(f16),
        "wq": wq.astype(f16),
        "wk": wk.astype(f16),
        "wv": wv.astype(f16),
        "wo": wo.astype(f16),
        "mcT": mcT.astype(f16),
        "rcT": rcT.astype(f16),
        "peb": pe_b.reshape(1, DEC).astype(f16),
        "b1": np.ascontiguousarray(b1.reshape(4, 128).T).astype(np.float32),
        "b2": np.ascontiguousarray(b2.reshape(4, 128).T).astype(np.float32),
        "psi": np.array([[psi_g[0], psi_b[0]]], np.float32),
    }
    in_maps = []
    for c in range(N_CORES):
        m = dict(shared)
        m["dec"] = np.ascontiguousarray(
            decoder[c].reshape(CIN, NPR, P, NPR, P).transpose(0, 2, 4, 1, 3)
            .reshape(CIN, 64, NPAT)).astype(f16)
        m["transT"] = np.ascontiguousarray(trans[c].T).astype(f16)
        in_maps.append(m)
    return in_maps


_NC_CACHE: dict = {}


def get_nc(repeat: int = 1):
    if repeat not in _NC_CACHE:
        _NC_CACHE[repeat] = build_nc(repeat)
    return _NC_CACHE[repeat]


def kernel(**inputs) -> np.ndarray:
    nc = get_nc(1)
    in_maps = prepare_in_maps(inputs)
    res = run_bass_kernel_spmd(nc, in_maps, core_ids=list(range(N_CORES)))
    outs = []
    for c in range(N_CORES):
        oq = res.results[c]["out"].reshape(SKIP, P, P, NPR, NPR)
        outs.append(oq.transpose(0, 3, 1, 4, 2).reshape(SKIP, IMG, IMG))
    return np.stack(outs).astype(np.float32)


if __name__ == "__main__":
    import jax

    sys.path.insert(0, "/root/problem")
    import reference

    inputs = {k: np.asarray(v) for k, v in reference.setup_inputs().items()}
    out = kernel(**inputs)
    print("out shape", out.shape, out.dtype)
